# revision 14
# baseline (speedup 1.0000x reference)
# Distributed GNN message-passing kernel for one TRN2 chip (8 NeuronCores).
#
# Reference model: 2x SAGEConv(mean) + 1x GraphConv(sum) + linear head +
# softmax over a width-1 axis. N=50000 nodes, E=800000 edges, D=H=64.
#
# Strategy (graph/data parallel, per the sharding hint):
#  - Nodes are sharded contiguously across the 8 cores (6250/core, padded to
#    6272 = 49 blocks of 128). Edges are assigned to the core owning their
#    destination and destination-sorted.
#  - Activations live feature-major in SBUF: hT [65, 6272] (row 64 = ones so
#    the bias rides inside the self-term matmul).
#  - Per layer: t = h @ Wl.T is computed per-shard (PE), DMAed to DRAM and
#    AllGathered so every core holds t for all nodes (the halo exchange).
#  - Aggregation: edges are processed in 128-edge tiles. A dma_gather pulls
#    t[src] rows (256B each) into SBUF edge-major ([128 edges, 64] per tile).
#    The scatter-add is a PE matmul: psum[64f,128n] += msg.T @ M where
#    M[e, j] = (dstloc[e] == j) * scale[e] is built in one DVE tensor_scalar
#    (is_equal, mult) against a resident iota tile. scale = 1/deg for the
#    mean layers, 1 for the sum layer, 0 for padding edges.
#  - dma_gather indices are int16, so sources are split into "lo" rows
#    (< 32768) and "hi" rows (>= 32768, gathered against an offset view).
#  - The self term (h @ Wr.T + b) accumulates into the same PSUM tile with
#    lhsT = [Wr.T; b] and rhs = hT[0:65, block]. ReLU (ScalarE) writes the
#    next layer's hT directly, so no transposes exist anywhere.
#  - Head: logits = h3 @ Wlin.T + blin per block; softmax over the width-1
#    axis is exp(x - max)/sum = exp(0)/1, computed as Exp(scale=0).
import functools
import numpy as np

N = 50000
E = 800000
D = 64
NCORES = 8
SHARD = N // NCORES              # 6250
BLK = 128
NBLK = (SHARD + BLK - 1) // BLK  # 49
SHARD_PAD = NBLK * BLK           # 6272
NAG = NCORES * SHARD_PAD         # 50176
SPLIT = 32768                    # int16 gather index limit
CHUNK_BLKS = 7                   # blocks per gather chunk (49 = 7*7)
GATHER_TILES = 8                 # max 128-edge tiles per dma_gather call.
                                 # The SWDGE descriptor ring holds
                                 # dynamic_dma_scratch_size//16 descriptors; a
                                 # call with more indices than ring capacity
                                 # wedges the device (that was the (1024,2048]
                                 # wedge with the default 16 KiB scratch).
DMA_SCRATCH = 16384              # SBUF bytes/partition for the SWDGE ring
NQUEUES = 3                      # SWDGE queues: 3 measured fastest (2.5ns/idx
                                 # vs 8.5 at 1); 4 queues crashes the runtime

_DEBUG_H3 = False  # when True, kernel also returns per-core hT3 [64, SHARD_PAD]


def _agrow(src):
    # row of node `src` inside the all-gathered tensor [NCORES*SHARD_PAD, 64]
    return (src // SHARD) * SHARD_PAD + (src % SHARD)


def _preprocess(edge_index):
    """Host-side index preprocessing. Returns per-core arrays + the shared
    static tile structure (identical across cores so the SPMD program is
    uniform)."""
    src = np.asarray(edge_index[0], dtype=np.int64)
    dst = np.asarray(edge_index[1], dtype=np.int64)
    deg = np.bincount(dst, minlength=N).astype(np.float64)
    invdeg = (1.0 / np.maximum(deg, 1.0)).astype(np.float32)

    rows = _agrow(src)
    core_of = dst // SHARD

    # per (core, block, phase) edge lists
    per_core = []
    for k in range(NCORES):
        m = core_of == k
        s_k = rows[m]
        d_k = dst[m] - k * SHARD
        order = np.argsort(d_k, kind="stable")
        s_k = s_k[order]
        d_k = d_k[order]
        blk_k = d_k >> 7
        lists = []
        for b in range(NBLK):
            mb = blk_k == b
            s_b = s_k[mb]
            d_b = d_k[mb] - b * BLK
            lo = s_b < SPLIT
            lists.append(
                (
                    (s_b[lo], d_b[lo]),
                    (s_b[~lo] - SPLIT, d_b[~lo]),
                )
            )
        per_core.append(lists)

    # uniform tile counts per (block, phase): max over cores
    ntl = np.zeros(NBLK, dtype=np.int64)
    nth = np.zeros(NBLK, dtype=np.int64)
    for b in range(NBLK):
        for k in range(NCORES):
            (slo, _), (shi, _) = per_core[k][b]
            ntl[b] = max(ntl[b], (len(slo) + BLK - 1) // BLK)
            nth[b] = max(nth[b], (len(shi) + BLK - 1) // BLK)
        ntl[b] = max(ntl[b], 1)  # keep >=1 so structure is simple
        nth[b] = max(nth[b], 1)

    TL = int(ntl.sum())
    TH = int(nth.sum())

    def build_core(k):
        idx_lo = np.zeros(TL * BLK, dtype=np.int16)
        idx_hi = np.zeros(TH * BLK, dtype=np.int16)
        dl_lo = np.zeros(TL * BLK, dtype=np.float32)
        dl_hi = np.zeros(TH * BLK, dtype=np.float32)
        scm_lo = np.zeros(TL * BLK, dtype=np.float32)
        scm_hi = np.zeros(TH * BLK, dtype=np.float32)
        sc1_lo = np.zeros(TL * BLK, dtype=np.float32)
        sc1_hi = np.zeros(TH * BLK, dtype=np.float32)
        off_l = 0
        off_h = 0
        for b in range(NBLK):
            (slo, dlo), (shi, dhi) = per_core[k][b]
            invd_lo = invdeg[dlo + b * BLK + k * SHARD]
            invd_hi = invdeg[dhi + b * BLK + k * SHARD]
            n = len(slo)
            idx_lo[off_l : off_l + n] = slo
            dl_lo[off_l : off_l + n] = dlo
            scm_lo[off_l : off_l + n] = invd_lo
            sc1_lo[off_l : off_l + n] = 1.0
            off_l += int(ntl[b]) * BLK
            n = len(shi)
            idx_hi[off_h : off_h + n] = shi
            dl_hi[off_h : off_h + n] = dhi
            scm_hi[off_h : off_h + n] = invd_hi
            sc1_hi[off_h : off_h + n] = 1.0
            off_h += int(nth[b]) * BLK

        def wrap_idx(a):
            # edge j lives at [j%16, j//16]; replicate the 16 rows to 128
            w = a.reshape(-1, 16).T  # [16, cols]
            return np.ascontiguousarray(np.tile(w, (8, 1)))  # [128, cols]

        def col_mat(a, T):
            # edge j of tile t at [j%128, t]
            return np.ascontiguousarray(a.reshape(T, BLK).T)

        return {
            "idx_lo": wrap_idx(idx_lo),
            "idx_hi": wrap_idx(idx_hi),
            "dl_lo": col_mat(dl_lo, TL),
            "dl_hi": col_mat(dl_hi, TH),
            "scm_lo": col_mat(scm_lo, TL),
            "scm_hi": col_mat(scm_hi, TH),
            "sc1_lo": col_mat(sc1_lo, TL),
            "sc1_hi": col_mat(sc1_hi, TH),
        }

    cores = [build_core(k) for k in range(NCORES)]
    return cores, tuple(int(x) for x in ntl), tuple(int(x) for x in nth)


@functools.lru_cache(maxsize=16)
def _compile(ntl, nth, debug_h3, repeat=1, variant="", gather_tiles=None,
             scratch=None, nqueues=None):
    # variant: comma-set of {"nocc", "nogather", "nom"} — timing-only ablations
    import concourse.bass as bass
    import concourse.mybir as mybir
    from concourse import bacc, tile
    from concourse.bass import IndirectOffsetOnAxis  # noqa: F401 (doc)

    gather_tiles = GATHER_TILES if gather_tiles is None else gather_tiles
    scratch = DMA_SCRATCH if scratch is None else scratch
    nqueues = NQUEUES if nqueues is None else nqueues
    # Ring capacity is 1024 descriptors in ucode regardless of the SBUF
    # carveout size — a 1536-idx call wedged the device even with
    # dynamic_dma_scratch_size=32768. 1024 idxs/call is the hard max.
    assert gather_tiles * BLK <= 1024, f"{gather_tiles*BLK} idxs/call wedges"

    dt = mybir.dt
    ntl = list(ntl)
    nth = list(nth)
    TL = sum(ntl)
    TH = sum(nth)

    nc = bacc.Bacc(
        "TRN2",
        target_bir_lowering=False,
        num_devices=NCORES,
        dynamic_dma_scratch_size=scratch,
        num_swdge_queues=nqueues,
    )

    # ---- DRAM parameters -------------------------------------------------
    xT_d = nc.dram_tensor("xT", [D, SHARD_PAD], dt.float32, kind="ExternalInput")
    idx_lo_d = nc.dram_tensor("idx_lo", [128, TL * 8], dt.int16, kind="ExternalInput")
    idx_hi_d = nc.dram_tensor("idx_hi", [128, TH * 8], dt.int16, kind="ExternalInput")
    dl_lo_d = nc.dram_tensor("dl_lo", [128, TL], dt.float32, kind="ExternalInput")
    dl_hi_d = nc.dram_tensor("dl_hi", [128, TH], dt.float32, kind="ExternalInput")
    scm_lo_d = nc.dram_tensor("scm_lo", [128, TL], dt.float32, kind="ExternalInput")
    scm_hi_d = nc.dram_tensor("scm_hi", [128, TH], dt.float32, kind="ExternalInput")
    sc1_lo_d = nc.dram_tensor("sc1_lo", [128, TL], dt.float32, kind="ExternalInput")
    sc1_hi_d = nc.dram_tensor("sc1_hi", [128, TH], dt.float32, kind="ExternalInput")
    iota_d = nc.dram_tensor("iota", [128, 128], dt.float32, kind="ExternalInput")
    wl_d = nc.dram_tensor("wl", [64, 3 * 64], dt.float32, kind="ExternalInput")
    wra_d = nc.dram_tensor("wra", [65, 3 * 64], dt.float32, kind="ExternalInput")
    whead_d = nc.dram_tensor("whead", [65, 1], dt.float32, kind="ExternalInput")
    out_d = nc.dram_tensor("out", [SHARD_PAD, 1], dt.float32, kind="ExternalOutput")
    if debug_h3:
        hdbg_d = nc.dram_tensor(
            "hdbg", [64, SHARD_PAD], dt.float32, kind="ExternalOutput"
        )

    # internal DRAM
    t_loc = nc.dram_tensor("t_loc", [SHARD_PAD, D], dt.float32)
    t_ags = [
        nc.dram_tensor(f"t_ag{l}", [NAG, D], dt.float32, addr_space="Shared")
        for l in range(3 * repeat)
    ]

    # chunk structure
    chunk_blocks = [
        list(range(c, min(c + CHUNK_BLKS, NBLK))) for c in range(0, NBLK, CHUNK_BLKS)
    ]
    tile_off_lo = np.concatenate([[0], np.cumsum(ntl)]).astype(int)
    tile_off_hi = np.concatenate([[0], np.cumsum(nth)]).astype(int)
    max_tl_chunk = max(sum(ntl[b] for b in cb) for cb in chunk_blocks)
    max_th_chunk = max(sum(nth[b] for b in cb) for cb in chunk_blocks)

    from contextlib import ExitStack

    with tile.TileContext(nc) as tc, ExitStack() as ctx:
        pool_const = ctx.enter_context(tc.tile_pool(name="const", bufs=1))
        pool_h = ctx.enter_context(tc.tile_pool(name="hstate", bufs=1))
        pool_g = ctx.enter_context(tc.tile_pool(name="gather", bufs=2))
        pool_m = ctx.enter_context(tc.tile_pool(name="onehot", bufs=4))
        pool_ps_agg = ctx.enter_context(tc.tile_pool(name="psagg", bufs=2, space="PSUM"))
        pool_ps_misc = ctx.enter_context(
            tc.tile_pool(name="psmisc", bufs=2, space="PSUM")
        )

        # ---- resident constants -----------------------------------------
        def load_const(name, dram, shape, dtype):
            t = pool_const.tile(shape, dtype, tag=name, name=name)
            nc.sync.dma_start(t[:], dram.ap())
            return t

        iota_sb = load_const("iota", iota_d, [128, 128], dt.float32)
        if "nogather" in variant or "nom" in variant:
            # timing-ablation stand-ins so skipped producers leave no
            # read-but-never-written tiles for the sim to reject
            dummy_g = pool_const.tile([128, 64], dt.float32, tag="dummy_g")
            dummy_m = pool_const.tile([128, 128], dt.float32, tag="dummy_m")
            nc.vector.memset(dummy_g[:], 0.0)
            nc.vector.memset(dummy_m[:], 0.0)
        wl_sb = load_const("wl", wl_d, [64, 3 * 64], dt.float32)
        wra_sb = load_const("wra", wra_d, [65, 3 * 64], dt.float32)
        whead_sb = load_const("whead", whead_d, [65, 1], dt.float32)
        idx_lo_sb = load_const("idx_lo", idx_lo_d, [128, TL * 8], dt.int16)
        idx_hi_sb = load_const("idx_hi", idx_hi_d, [128, TH * 8], dt.int16)
        dl_lo_sb = load_const("dl_lo", dl_lo_d, [128, TL], dt.float32)
        dl_hi_sb = load_const("dl_hi", dl_hi_d, [128, TH], dt.float32)
        scm_lo_sb = load_const("scm_lo", scm_lo_d, [128, TL], dt.float32)
        scm_hi_sb = load_const("scm_hi", scm_hi_d, [128, TH], dt.float32)
        sc1_lo_sb = load_const("sc1_lo", sc1_lo_d, [128, TL], dt.float32)
        sc1_hi_sb = load_const("sc1_hi", sc1_hi_d, [128, TH], dt.float32)

        # ---- activation state (feature-major, ones row at 64) -----------
        hT = [
            pool_h.tile([65, SHARD_PAD], dt.float32, tag=f"hT{i}", name=f"hT{i}")
            for i in range(2)
        ]
        nc.vector.memset(hT[0][64:65, :], 1.0)
        nc.vector.memset(hT[1][64:65, :], 1.0)

        t_stage = pool_h.tile([128, NBLK * 64], dt.float32, tag="tstage")
        out_stage = pool_h.tile([128, NBLK], dt.float32, tag="ostage")

        t_loc_v = t_loc.ap().rearrange("(b p) f -> p b f", p=128)

        gather_q = [0]  # running gather-call counter for queue round-robin
        rep_layers = [(rep, l) for rep in range(repeat) for l in range(3)]
        for rep, l in rep_layers:
            if l == 0:
                nc.sync.dma_start(hT[0][0:64, :], xT_d.ap())
            h_cur = hT[l % 2]
            h_nxt = hT[(l + 1) % 2]
            scm_sb = (scm_lo_sb, scm_hi_sb) if l < 2 else (sc1_lo_sb, sc1_hi_sb)

            # t = h @ Wl.T (node-major), staged then one DMA + AllGather
            for b in range(NBLK):
                ps = pool_ps_misc.tile([128, 64], dt.float32, tag="pst")
                nc.tensor.matmul(
                    ps[:],
                    lhsT=h_cur[0:64, b * BLK : (b + 1) * BLK],
                    rhs=wl_sb[:, l * 64 : (l + 1) * 64],
                    start=True,
                    stop=True,
                )
                nc.scalar.copy(t_stage[:, b * 64 : (b + 1) * 64], ps[:])
            nc.sync.dma_start(
                t_loc_v, t_stage[:].rearrange("p (b f) -> p b f", f=64)
            )
            if "nocc" not in variant:
                nc.gpsimd.collective_compute(
                    "AllGather",
                    mybir.AluOpType.bypass,
                    replica_groups=[list(range(NCORES))],
                    ins=[t_loc.ap()],
                    outs=[t_ags[3 * rep + l].ap()],
                )
            t_ag_lo = t_ags[3 * rep + l].ap()
            t_ag_hi = t_ags[3 * rep + l].ap()[SPLIT:, :]

            # aggregation over chunks of blocks
            for cb in chunk_blocks:
                c_lo0 = tile_off_lo[cb[0]]
                c_hi0 = tile_off_hi[cb[0]]
                n_tl = sum(ntl[b] for b in cb)
                n_th = sum(nth[b] for b in cb)
                if "nogather" in variant:
                    glo = ghi = None
                else:
                    glo = pool_g.tile([128, max_tl_chunk, 64], dt.float32, tag="glo")
                    ghi = pool_g.tile([128, max_th_chunk, 64], dt.float32, tag="ghi")
                for g_t, src_ap, idx_sb, off0, n_t in (
                    (glo, t_ag_lo, idx_lo_sb, c_lo0, n_tl),
                    (ghi, t_ag_hi, idx_hi_sb, c_hi0, n_th),
                ):
                    for j0 in range(0, n_t, gather_tiles):
                        if "nogather" in variant:
                            break
                        j1 = min(j0 + gather_tiles, n_t)
                        nc.gpsimd.dma_gather(
                            g_t[:, j0:j1, :],
                            src_ap,
                            idx_sb[:, (off0 + j0) * 8 : (off0 + j1) * 8],
                            num_idxs=(j1 - j0) * BLK,
                            num_idxs_reg=(j1 - j0) * BLK,
                            elem_size=64,
                            queue_num=gather_q[0] % nqueues,
                        )
                        gather_q[0] += 1
                for b in cb:
                    ps = pool_ps_agg.tile([64, 128], dt.float32, tag="psagg")
                    # self term + bias: [Wr.T; b].T @ hT[0:65, blk]
                    nc.tensor.matmul(
                        ps[:],
                        lhsT=wra_sb[:, l * 64 : (l + 1) * 64],
                        rhs=h_cur[:, b * BLK : (b + 1) * BLK],
                        start=True,
                        stop=False,
                    )
                    n_parts = ntl[b] + nth[b]
                    done = 0
                    for phase, (g_t, off0, sb_dl, sb_sc, cnt) in enumerate(
                        (
                            (glo, c_lo0, dl_lo_sb, scm_sb[0], ntl[b]),
                            (ghi, c_hi0, dl_hi_sb, scm_sb[1], nth[b]),
                        )
                    ):
                        t0 = (tile_off_lo[b] if phase == 0 else tile_off_hi[b])
                        for t in range(cnt):
                            g = t0 + t  # global tile index (per phase)
                            tc_i = g - off0  # tile index within the chunk
                            if "nom" in variant:
                                m = dummy_m
                            else:
                                m = pool_m.tile([128, 128], dt.float32, tag="m")
                                nc.vector.tensor_scalar(
                                    m[:],
                                    iota_sb[:],
                                    sb_dl[:, g : g + 1],
                                    sb_sc[:, g : g + 1],
                                    mybir.AluOpType.is_equal,
                                    mybir.AluOpType.mult,
                                )
                            done += 1
                            nc.tensor.matmul(
                                ps[:],
                                lhsT=(
                                    dummy_g[:]
                                    if "nogather" in variant
                                    else g_t[:, tc_i, :]
                                ),
                                rhs=m[:],
                                start=False,
                                stop=(done == n_parts),
                            )
                    # ReLU -> next layer's feature-major state
                    nc.scalar.activation(
                        h_nxt[0:64, b * BLK : (b + 1) * BLK],
                        ps[:],
                        mybir.ActivationFunctionType.Relu,
                    )

            if l == 2:
                # ---- head: softmax over width-1 axis == exp(0)/1 --------
                h_fin = h_nxt
                for b in range(NBLK):
                    ps = pool_ps_misc.tile([128, 1], dt.float32, tag="pst")
                    nc.tensor.matmul(
                        ps[:],
                        lhsT=h_fin[:, b * BLK : (b + 1) * BLK],
                        rhs=whead_sb[:],
                        start=True,
                        stop=True,
                    )
                    nc.scalar.activation(
                        out_stage[:, b : b + 1],
                        ps[:],
                        mybir.ActivationFunctionType.Exp,
                        scale=0.0,
                    )
                nc.sync.dma_start(
                    out_d.ap().rearrange("(b p) one -> p (b one)", p=128),
                    out_stage[:],
                )
                if debug_h3:
                    nc.sync.dma_start(hdbg_d.ap(), h_fin[0:64, :])

    nc.compile()
    return nc


def _pack_weights(Wl1, Wr1, b1, Wl2, Wr2, b2, Wrel3, Wroot3, b3, Wlin, blin):
    wl = np.concatenate(
        [np.ascontiguousarray(W.T) for W in (Wl1, Wl2, Wrel3)], axis=1
    ).astype(np.float32)  # [64, 192]
    wra = np.concatenate(
        [
            np.concatenate([W.T, b[None, :]], axis=0)
            for W, b in ((Wr1, b1), (Wr2, b2), (Wroot3, b3))
        ],
        axis=1,
    ).astype(np.float32)  # [65, 192]
    whead = np.concatenate([Wlin.T, blin[None, :]], axis=0).astype(np.float32)
    return wl, wra, whead


def kernel(
    x,
    Wl1,
    Wr1,
    b1,
    Wl2,
    Wr2,
    b2,
    Wrel3,
    Wroot3,
    b3,
    Wlin,
    blin,
    edge_index,
):
    from concourse.bass_utils import run_bass_kernel_spmd

    x = np.asarray(x, dtype=np.float32)
    edge_index = np.asarray(edge_index)
    cores, ntl, nth = _preprocess(edge_index)
    nc = _compile(ntl, nth, _DEBUG_H3)

    wl, wra, whead = _pack_weights(
        np.asarray(Wl1, np.float32),
        np.asarray(Wr1, np.float32),
        np.asarray(b1, np.float32),
        np.asarray(Wl2, np.float32),
        np.asarray(Wr2, np.float32),
        np.asarray(b2, np.float32),
        np.asarray(Wrel3, np.float32),
        np.asarray(Wroot3, np.float32),
        np.asarray(b3, np.float32),
        np.asarray(Wlin, np.float32),
        np.asarray(blin, np.float32),
    )
    iota = np.broadcast_to(np.arange(128, dtype=np.float32), (128, 128)).copy()

    in_maps = []
    for k in range(NCORES):
        xT = np.zeros((D, SHARD_PAD), dtype=np.float32)
        xT[:, :SHARD] = x[k * SHARD : (k + 1) * SHARD].T
        m = dict(cores[k])
        m.update(
            xT=np.ascontiguousarray(xT),
            iota=iota,
            wl=wl,
            wra=wra,
            whead=whead,
        )
        in_maps.append(m)

    res = run_bass_kernel_spmd(nc, in_maps, list(range(NCORES)))
    out = np.empty((N, 1), dtype=np.float32)
    for k in range(NCORES):
        out[k * SHARD : (k + 1) * SHARD] = res.results[k]["out"][:SHARD]
    kernel._res = res
    if _DEBUG_H3:
        kernel._h3 = np.concatenate(
            [res.results[k]["hdbg"][:, :SHARD].T for k in range(NCORES)], axis=0
        )
    return out

